# revision 16
# baseline (speedup 1.0000x reference)
"""Trainium2 Bass kernel for per-batch channel attention (CxAM-style).

Reference (per batch element b):
    q = (Wq @ x_b + bq)        # [64, T]
    k = (Wk @ x_b + bk)        # [64, T]
    v = (Wv @ x_b + bv)        # [512, T]
    R = q.T @ k                # [T, T]
    A = softmax(R, axis=-1)
    out_b = v @ A.T            # [512, T]

Sharding: pure data-parallel - batch B=8, one batch element per NeuronCore.

Key layout decisions:
  - All weight transposes happen on HOST (numpy) - the device receives
    wqkT [ch,ci,qk-out] and wvT [ch,ci,c] ready for matmul, plus x
    pre-cast to bf16 and pre-tiled as [128, tq, ci, 512] so every DMA is
    128 fat contiguous descriptors. x in bf16 halves HBM traffic.
  - x streams in 4 t-slabs; QK and V projections chase the slabs.
  - qk packs Q(rows 0:64)/K(64:128); kq is the swapped copy built by
    ACT Identity+bias reads of the projection PSUM at shifted base.
  - Scores S^T[s,t] per 128-s-chunk (contract 64); exp on ACT into bf16.
  - A.T@V via accumulating matmuls: stationary vT chunk, moving exp tile.
  - Softmax denominator: ones[128,128] bf16 stationary matmuls reduce the
    exp tiles AND land the result pre-broadcast across partitions, so the
    epilogue is just reciprocal + multiply (no [1,N] ops, no broadcast
    matmul chain). Keeping DVE/GPSIMD traffic minimal matters: heavy
    concurrent engine activity slows every PE matmul ~15-25%.
"""

import os

os.environ.setdefault("MYCRO_LOCAL_CACHE", "1")

import numpy as np

import concourse.bass as bass
import concourse.mybir as mybir
import concourse.tile as tile
from concourse import bacc
from concourse.bass_utils import run_bass_kernel_spmd

F32 = mybir.dt.float32
F32R = mybir.dt.float32r
BF16 = mybir.dt.bfloat16
AF = mybir.ActivationFunctionType

B = 8
C = 512
T = 2048
CQ = 64
NCORES = 8

TB = 512            # t-block / slab width
NTB = T // TB       # 4
NSC = T // 128      # 16 s-chunks
NPAIR = NSC // 2    # 8 score pairs per t-block
NCH = C // 128      # 4 contraction chunks
NCC = C // 128      # 4 output channel chunks


def _build_program() -> bass.Bass:
    nc = bacc.Bacc("TRN2", target_bir_lowering=False, debug=False, num_devices=NCORES)

    x_d = nc.declare_dram_parameter("x", [128, NTB, NCH, TB], BF16, isOutput=False)
    wqkT_d = nc.declare_dram_parameter("wqkT", [128, NCH, 128], BF16, isOutput=False)
    wvT_d = nc.declare_dram_parameter("wvT", [128, NCH, C], BF16, isOutput=False)
    bqk_d = nc.declare_dram_parameter("bqk", [128, 1], F32, isOutput=False)
    bv_d = nc.declare_dram_parameter("bv", [128, C], F32, isOutput=False)
    out_d = nc.declare_dram_parameter("out", [C, T], F32, isOutput=True)

    with tile.TileContext(nc) as tc:
        with (
            tc.tile_pool(name="const", bufs=1) as const,
            tc.tile_pool(name="weights", bufs=1) as wpool,
        ):
            # ---- constants
            ones_bb = const.tile([128, 128], BF16)
            nc.gpsimd.memset(ones_bb[:], 1.0)

            # ---- inputs -> SBUF (x streamed per t-slab; slab 0 issued
            # first so its transfer starts before the small params queue)
            x_bf = wpool.tile([128, NTB, NCH, TB], BF16)
            wvT = wpool.tile([128, NCH, C], BF16)
            wqkT = wpool.tile([128, NCH, 128], BF16)
            bqk = wpool.tile([128, 1], F32)
            bv_bcast = wpool.tile([128, C], F32)
            nc.sync.dma_start(out=x_bf[:, 0], in_=x_d[:, 0])
            nc.sync.dma_start(out=wqkT[:], in_=wqkT_d[:])
            nc.sync.dma_start(out=bqk[:], in_=bqk_d[:])
            nc.sync.dma_start(out=bv_bcast[:], in_=bv_d[:])
            nc.sync.dma_start(out=wvT[:], in_=wvT_d[:])
            for tq in range(1, NTB):
                nc.sync.dma_start(out=x_bf[:, tq], in_=x_d[:, tq])

            qk = wpool.tile([128, T], BF16)   # rows 0:64 Q, 64:128 K
            kq = wpool.tile([128, T], BF16)   # rows 0:64 K, 64:128 Q
            vT = wpool.tile([128, NSC, C], BF16)

            # score PSUM pool opened BEFORE the projection pool: its banks
            # are carved out up front, so the first score matmuls never wait
            # on the projection pool's close barrier
            sc_pool_cm = tc.tile_pool(name="ps_sc", bufs=1, space="PSUM")
            ps_sc = sc_pool_cm.__enter__()

            with (
                tc.tile_pool(name="ps_proj", bufs=1, space="PSUM") as ps_proj,
            ):
                for tq in range(NTB):
                    tsl = slice(tq * TB, (tq + 1) * TB)
                    # packed Q/K projection for this slab
                    ps = ps_proj.tile([128, TB], F32, tag="qkp", bufs=2, name=f"qkp_{tq}")
                    for ci in range(NCH):
                        nc.tensor.matmul(
                            ps[:],
                            wqkT[:, ci, :],
                            x_bf[:, tq, ci, :],
                            start=(ci == 0),
                            stop=(ci == NCH - 1),
                        )
                    nc.scalar.activation(qk[:, tsl], ps[:], AF.Identity, bias=bqk[:, 0:1])
                    # swapped copy: kq rows 0:64 = K, 64:128 = Q
                    nc.scalar.activation(kq[0:CQ, tsl], ps[CQ:128, :], AF.Identity, bias=bqk[CQ:128, 0:1])
                    nc.scalar.activation(kq[CQ:128, tsl], ps[0:CQ, :], AF.Identity, bias=bqk[0:CQ, 0:1])

                    # V^T projection for the 4 s-chunks of this slab
                    for jj in range(NCH):
                        j = NCH * tq + jj
                        psv = ps_proj.tile([128, C], F32, tag="vp", bufs=4, name=f"vp_{j}")
                        for ci in range(NCH):
                            nc.tensor.matmul(
                                psv[:],
                                x_bf[:, tq, ci, jj * 128:(jj + 1) * 128],
                                wvT[:, ci, :],
                                start=(ci == 0),
                                stop=(ci == NCH - 1),
                            )
                        nc.vector.tensor_add(vT[:, j, :], psv[:], bv_bcast[:])

            # ---- main attention loop, software-pipelined one pair deep
            with (
                tc.tile_pool(name="et", bufs=4) as et_pool,
                tc.tile_pool(name="ps_av", bufs=1, space="PSUM") as ps_av,
                tc.tile_pool(name="ps_dn", bufs=1, space="PSUM") as ps_dn,
                tc.tile_pool(name="small", bufs=2) as small,
                tc.tile_pool(name="outp", bufs=2) as outp,
            ):
                avs = {}
                dnbs = {}

                def start_block(tb):
                    avs[tb] = [
                        ps_av.tile([128, TB], F32, tag=f"av{ck}", name=f"av{ck}_{tb}")
                        for ck in range(NCC)
                    ]
                    dnbs[tb] = ps_dn.tile([128, TB], F32, tag="dnb", name=f"dnb_{tb}")

                def emit_scores(tb, jp):
                    tsl = slice(tb * TB, (tb + 1) * TB)
                    j0, j1 = 2 * jp, 2 * jp + 1
                    etp0 = et_pool.tile([128, TB], BF16, tag="etp0", name=f"etp0_{tb}_{jp}")
                    etp1 = et_pool.tile([128, TB], BF16, tag="etp1", name=f"etp1_{tb}_{jp}")
                    sc0 = ps_sc.tile([128, TB], F32, tag="sc0", name=f"sc0_{tb}_{jp}")
                    nc.tensor.matmul(
                        sc0[:],
                        kq[0:CQ, j0 * 128:(j0 + 1) * 128],
                        qk[0:CQ, tsl],
                        start=True,
                        stop=True,
                    )
                    sc1 = ps_sc.tile([128, TB], F32, tag="sc1", name=f"sc1_{tb}_{jp}")
                    nc.tensor.matmul(
                        sc1[:],
                        qk[CQ:128, j1 * 128:(j1 + 1) * 128],
                        kq[CQ:128, tsl],
                        start=True,
                        stop=True,
                        tile_position=(64, 0),
                    )
                    nc.scalar.activation(etp0[:], sc0[:], AF.Exp)
                    nc.scalar.activation(etp1[:], sc1[:], AF.Exp)
                    return (etp0, etp1)

                def emit_consume(tb, jp, etp):
                    for idx in (0, 1):
                        j = 2 * jp + idx
                        # denominator: ones[128,128] stationary makes the
                        # reduction land pre-broadcast across partitions
                        nc.tensor.matmul(
                            dnbs[tb][:],
                            ones_bb[:],
                            etp[idx][:],
                            start=(j == 0),
                            stop=(j == NSC - 1),
                        )
                        for ck in range(NCC):
                            nc.tensor.matmul(
                                avs[tb][ck][:],
                                vT[:, j, ck * 128:(ck + 1) * 128],
                                etp[idx][:],
                                start=(j == 0),
                                stop=(j == NSC - 1),
                            )

                def finish_block(tb):
                    tsl = slice(tb * TB, (tb + 1) * TB)
                    rcp = small.tile([128, TB], F32, tag="rcp", name=f"rcp_{tb}")
                    nc.vector.reciprocal_approx_fast(rcp[:], dnbs[tb][:])

                    for ck in range(NCC):
                        ot = outp.tile(
                            [128, TB], F32, tag=f"ot{ck}", name=f"ot{ck}_{tb}"
                        )
                        nc.vector.tensor_mul(ot[:], avs[tb][ck][:], rcp[:])
                        nc.sync.dma_start(
                            out=out_d[ck * 128:(ck + 1) * 128, tsl], in_=ot[:]
                        )

                pending = None  # (tb, jp, etp)
                for tb in range(NTB):
                    for jp in range(NPAIR):
                        etp = emit_scores(tb, jp)
                        if jp == 0:
                            # after the first scores: sc banks get the
                            # earliest-freed projection banks
                            start_block(tb)
                        if pending is not None:
                            ptb, pjp, petp = pending
                            emit_consume(ptb, pjp, petp)
                            if pjp == NPAIR - 1:
                                finish_block(ptb)
                        pending = (tb, jp, etp)
                ptb, pjp, petp = pending
                emit_consume(ptb, pjp, petp)
                finish_block(ptb)
            sc_pool_cm.__exit__(None, None, None)

    nc.compile()
    return nc


_PROGRAM = None


def _get_program() -> bass.Bass:
    global _PROGRAM
    if _PROGRAM is None:
        _PROGRAM = _build_program()
    return _PROGRAM


def prep_inputs(inputs):
    """Host-side packing: transpose/tile weights, cast x to bf16 slabs."""
    import ml_dtypes

    bf16 = ml_dtypes.bfloat16
    x = np.asarray(inputs["x"], dtype=np.float32)
    wq = np.asarray(inputs["Wq"], dtype=np.float32)
    bq = np.asarray(inputs["bq"], dtype=np.float32)
    wk = np.asarray(inputs["Wk"], dtype=np.float32)
    bk = np.asarray(inputs["bk"], dtype=np.float32)
    wv = np.asarray(inputs["Wv"], dtype=np.float32)
    bv = np.asarray(inputs["bv"], dtype=np.float32)

    w8 = np.concatenate([wq, wk], axis=0)                     # [128, C]
    wqkT = np.ascontiguousarray(
        w8.T.reshape(NCH, 128, 128).transpose(1, 0, 2)
    ).astype(bf16)                                            # [128, ci, 128]
    wvT = np.ascontiguousarray(
        wv.T.reshape(NCH, 128, C).transpose(1, 0, 2)
    ).astype(bf16)                                            # [128, ci, C]
    bqk = np.concatenate([bq, bk], axis=0).reshape(128, 1)
    bvr = np.ascontiguousarray(np.broadcast_to(bv.reshape(1, C), (128, C)))

    in_maps = []
    for b in range(NCORES):
        xb = np.ascontiguousarray(
            x[b].reshape(NCH, 128, NTB, TB).transpose(1, 2, 0, 3)
        ).astype(bf16)                                        # [128, tq, ci, TB]
        in_maps.append(
            {"x": xb, "wqkT": wqkT, "wvT": wvT, "bqk": bqk, "bv": bvr}
        )
    return in_maps


def kernel(**inputs: np.ndarray) -> np.ndarray:
    nc = _get_program()
    in_maps = prep_inputs(inputs)
    res = run_bass_kernel_spmd(nc, in_maps, list(range(NCORES)))
    out = np.stack([res.results[b]["out"] for b in range(NCORES)], axis=0)
    return out.astype(np.float32)


if __name__ == "__main__":
    import reference

    inputs = {k: np.asarray(v) for k, v in reference.setup_inputs().items()}
    expected = np.asarray(reference.reference(**inputs))
    actual = kernel(**inputs)
    rel = np.linalg.norm(actual - expected) / np.linalg.norm(expected)
    print("Relative error:", rel)


# revision 17
# speedup vs baseline: 1.1696x; 1.1696x over previous
"""Trainium2 Bass kernel for per-batch channel attention (CxAM-style).

Reference (per batch element b):
    q = (Wq @ x_b + bq)        # [64, T]
    k = (Wk @ x_b + bk)        # [64, T]
    v = (Wv @ x_b + bv)        # [512, T]
    R = q.T @ k                # [T, T]
    A = softmax(R, axis=-1)
    out_b = v @ A.T            # [512, T]

Sharding: pure data-parallel - batch B=8, one batch element per NeuronCore.

Key layout decisions:
  - All weight transposes happen on HOST (numpy) - the device receives
    wqkT [ch,ci,qk-out] and wvT [ch,ci,c] ready for matmul, plus x
    pre-cast to bf16 and pre-tiled as [128, tq, ci, 512] so every DMA is
    128 fat contiguous descriptors. x in bf16 halves HBM traffic.
  - x streams in 4 t-slabs; QK and V projections chase the slabs.
  - qk packs Q(rows 0:64)/K(64:128); kq is the swapped copy built by
    ACT Identity+bias reads of the projection PSUM at shifted base.
  - Scores S^T[s,t] per 128-s-chunk (contract 64); exp on ACT into bf16.
  - A.T@V via accumulating matmuls: stationary vT chunk, moving exp tile.
  - Softmax denominator: ones[128,128] bf16 stationary matmuls reduce the
    exp tiles AND land the result pre-broadcast across partitions, so the
    epilogue is just reciprocal + multiply (no [1,N] ops, no broadcast
    matmul chain). Keeping DVE/GPSIMD traffic minimal matters: heavy
    concurrent engine activity slows every PE matmul ~15-25%.
"""

import os

os.environ.setdefault("MYCRO_LOCAL_CACHE", "1")

import numpy as np

import concourse.bass as bass
import concourse.mybir as mybir
import concourse.tile as tile
from concourse import bacc
from concourse.bass_utils import run_bass_kernel_spmd

F32 = mybir.dt.float32
F32R = mybir.dt.float32r
BF16 = mybir.dt.bfloat16
AF = mybir.ActivationFunctionType

B = 8
C = 512
T = 2048
CQ = 64
NCORES = 8

TB = 512            # t-block / slab width
NTB = T // TB       # 4
NSC = T // 128      # 16 s-chunks
NPAIR = NSC // 2    # 8 score pairs per t-block
NCH = C // 128      # 4 contraction chunks
NCC = C // 128      # 4 output channel chunks


def _build_program() -> bass.Bass:
    nc = bacc.Bacc("TRN2", target_bir_lowering=False, debug=False, num_devices=NCORES)

    x_d = nc.declare_dram_parameter("x", [128, NTB, NCH, TB], BF16, isOutput=False)
    wqkT_d = nc.declare_dram_parameter("wqkT", [128, NCH, 128], BF16, isOutput=False)
    wvT_d = nc.declare_dram_parameter("wvT", [128, NCH, C], BF16, isOutput=False)
    bqk_d = nc.declare_dram_parameter("bqk", [128, 1], F32, isOutput=False)
    bv_d = nc.declare_dram_parameter("bv", [1, C], F32, isOutput=False)
    out_d = nc.declare_dram_parameter("out", [C, T], F32, isOutput=True)

    with tile.TileContext(nc) as tc:
        with (
            tc.tile_pool(name="const", bufs=1) as const,
            tc.tile_pool(name="weights", bufs=1) as wpool,
        ):
            # ---- constants
            onesr_f = const.tile([1, 128], F32)
            nc.gpsimd.memset(onesr_f[:], 1.0)
            ones_row = const.tile([1, 128], F32R)
            nc.vector.tensor_copy(ones_row[:], onesr_f[:])
            ones_bb = const.tile([128, 128], BF16)
            nc.gpsimd.memset(ones_bb[:], 1.0)

            # ---- inputs -> SBUF (x streamed per t-slab; slab 0 issued
            # first so its transfer starts before the small params queue)
            x_bf = wpool.tile([128, NTB, NCH, TB], BF16)
            wvT = wpool.tile([128, NCH, C], BF16)
            wqkT = wpool.tile([128, NCH, 128], BF16)
            bqk = wpool.tile([128, 1], F32)
            bv_row = wpool.tile([1, C], F32)
            nc.sync.dma_start(out=x_bf[:, 0], in_=x_d[:, 0])
            nc.sync.dma_start(out=wqkT[:], in_=wqkT_d[:])
            nc.sync.dma_start(out=bqk[:], in_=bqk_d[:])
            nc.sync.dma_start(out=bv_row[:], in_=bv_d[:])
            nc.sync.dma_start(out=wvT[:], in_=wvT_d[:])
            for tq in range(1, NTB):
                nc.sync.dma_start(out=x_bf[:, tq], in_=x_d[:, tq])

            qk = wpool.tile([128, T], BF16)   # rows 0:64 Q, 64:128 K
            kq = wpool.tile([128, T], BF16)   # rows 0:64 K, 64:128 Q
            vT = wpool.tile([128, NSC, C], BF16)
            bv_bcast = wpool.tile([128, C], F32)

            with (
                tc.tile_pool(name="ps_proj", bufs=1, space="PSUM") as ps_proj,
            ):
                bv_r = wpool.tile([1, C], F32R)
                nc.vector.tensor_copy(bv_r[:], bv_row[:])

                for tq in range(NTB):
                    tsl = slice(tq * TB, (tq + 1) * TB)
                    # packed Q/K projection for this slab
                    ps = ps_proj.tile([128, TB], F32, tag="qkp", bufs=2, name=f"qkp_{tq}")
                    for ci in range(NCH):
                        nc.tensor.matmul(
                            ps[:],
                            wqkT[:, ci, :],
                            x_bf[:, tq, ci, :],
                            start=(ci == 0),
                            stop=(ci == NCH - 1),
                        )
                    nc.scalar.activation(qk[:, tsl], ps[:], AF.Identity, bias=bqk[:, 0:1])
                    # swapped copy: kq rows 0:64 = K, 64:128 = Q
                    nc.scalar.activation(kq[0:CQ, tsl], ps[CQ:128, :], AF.Identity, bias=bqk[CQ:128, 0:1])
                    nc.scalar.activation(kq[CQ:128, tsl], ps[0:CQ, :], AF.Identity, bias=bqk[0:CQ, 0:1])

                    if tq == 0:
                        # bv broadcast [1,C] -> [128,C] via f32r ones matmul,
                        # tucked behind the slab-0 QK projection
                        bvb = ps_proj.tile([128, C], F32, tag="bvb", bufs=1)
                        nc.tensor.matmul(bvb[:], ones_row[:], bv_r[:], start=True, stop=True)
                        nc.vector.tensor_copy(bv_bcast[:], bvb[:])

                    # V^T projection for the 4 s-chunks of this slab
                    for jj in range(NCH):
                        j = NCH * tq + jj
                        psv = ps_proj.tile([128, C], F32, tag="vp", bufs=4, name=f"vp_{j}")
                        for ci in range(NCH):
                            nc.tensor.matmul(
                                psv[:],
                                x_bf[:, tq, ci, jj * 128:(jj + 1) * 128],
                                wvT[:, ci, :],
                                start=(ci == 0),
                                stop=(ci == NCH - 1),
                            )
                        nc.vector.tensor_add(vT[:, j, :], psv[:], bv_bcast[:])

            # ---- main attention loop, software-pipelined one pair deep
            with (
                tc.tile_pool(name="et", bufs=4) as et_pool,
                tc.tile_pool(name="ps_sc", bufs=1, space="PSUM") as ps_sc,
                tc.tile_pool(name="ps_av", bufs=1, space="PSUM") as ps_av,
                tc.tile_pool(name="ps_dn", bufs=1, space="PSUM") as ps_dn,
                tc.tile_pool(name="small", bufs=2) as small,
                tc.tile_pool(name="outp", bufs=2) as outp,
            ):
                avs = {}
                dnbs = {}

                def start_block(tb):
                    avs[tb] = [
                        ps_av.tile([128, TB], F32, tag=f"av{ck}", name=f"av{ck}_{tb}")
                        for ck in range(NCC)
                    ]
                    dnbs[tb] = ps_dn.tile([128, TB], F32, tag="dnb", name=f"dnb_{tb}")

                def emit_scores(tb, jp):
                    tsl = slice(tb * TB, (tb + 1) * TB)
                    j0, j1 = 2 * jp, 2 * jp + 1
                    etp0 = et_pool.tile([128, TB], BF16, tag="etp0", name=f"etp0_{tb}_{jp}")
                    etp1 = et_pool.tile([128, TB], BF16, tag="etp1", name=f"etp1_{tb}_{jp}")
                    sc0 = ps_sc.tile([128, TB], F32, tag="sc0", name=f"sc0_{tb}_{jp}")
                    nc.tensor.matmul(
                        sc0[:],
                        kq[0:CQ, j0 * 128:(j0 + 1) * 128],
                        qk[0:CQ, tsl],
                        start=True,
                        stop=True,
                    )
                    sc1 = ps_sc.tile([128, TB], F32, tag="sc1", name=f"sc1_{tb}_{jp}")
                    nc.tensor.matmul(
                        sc1[:],
                        qk[CQ:128, j1 * 128:(j1 + 1) * 128],
                        kq[CQ:128, tsl],
                        start=True,
                        stop=True,
                        tile_position=(64, 0),
                    )
                    nc.scalar.activation(etp0[:], sc0[:], AF.Exp)
                    nc.scalar.activation(etp1[:], sc1[:], AF.Exp)
                    return (etp0, etp1)

                def emit_consume(tb, jp, etp):
                    for idx in (0, 1):
                        j = 2 * jp + idx
                        # denominator: ones[128,128] stationary makes the
                        # reduction land pre-broadcast across partitions
                        nc.tensor.matmul(
                            dnbs[tb][:],
                            ones_bb[:],
                            etp[idx][:],
                            start=(j == 0),
                            stop=(j == NSC - 1),
                        )
                        for ck in range(NCC):
                            nc.tensor.matmul(
                                avs[tb][ck][:],
                                vT[:, j, ck * 128:(ck + 1) * 128],
                                etp[idx][:],
                                start=(j == 0),
                                stop=(j == NSC - 1),
                            )

                def finish_block(tb):
                    tsl = slice(tb * TB, (tb + 1) * TB)
                    rcp = small.tile([128, TB], F32, tag="rcp", name=f"rcp_{tb}")
                    nc.vector.reciprocal_approx_fast(rcp[:], dnbs[tb][:])

                    for ck in range(NCC):
                        ot = outp.tile(
                            [128, TB], F32, tag=f"ot{ck}", name=f"ot{ck}_{tb}"
                        )
                        nc.vector.tensor_mul(ot[:], avs[tb][ck][:], rcp[:])
                        nc.sync.dma_start(
                            out=out_d[ck * 128:(ck + 1) * 128, tsl], in_=ot[:]
                        )

                pending = None  # (tb, jp, etp)
                for tb in range(NTB):
                    for jp in range(NPAIR):
                        etp = emit_scores(tb, jp)
                        if jp == 0:
                            # after the first scores: sc banks get the
                            # earliest-freed projection banks
                            start_block(tb)
                        if pending is not None:
                            ptb, pjp, petp = pending
                            emit_consume(ptb, pjp, petp)
                            if pjp == NPAIR - 1:
                                finish_block(ptb)
                        pending = (tb, jp, etp)
                ptb, pjp, petp = pending
                emit_consume(ptb, pjp, petp)
                finish_block(ptb)

    nc.compile()
    return nc


_PROGRAM = None


def _get_program() -> bass.Bass:
    global _PROGRAM
    if _PROGRAM is None:
        _PROGRAM = _build_program()
    return _PROGRAM


def prep_inputs(inputs):
    """Host-side packing: transpose/tile weights, cast x to bf16 slabs."""
    import ml_dtypes

    bf16 = ml_dtypes.bfloat16
    x = np.asarray(inputs["x"], dtype=np.float32)
    wq = np.asarray(inputs["Wq"], dtype=np.float32)
    bq = np.asarray(inputs["bq"], dtype=np.float32)
    wk = np.asarray(inputs["Wk"], dtype=np.float32)
    bk = np.asarray(inputs["bk"], dtype=np.float32)
    wv = np.asarray(inputs["Wv"], dtype=np.float32)
    bv = np.asarray(inputs["bv"], dtype=np.float32)

    w8 = np.concatenate([wq, wk], axis=0)                     # [128, C]
    wqkT = np.ascontiguousarray(
        w8.T.reshape(NCH, 128, 128).transpose(1, 0, 2)
    ).astype(bf16)                                            # [128, ci, 128]
    wvT = np.ascontiguousarray(
        wv.T.reshape(NCH, 128, C).transpose(1, 0, 2)
    ).astype(bf16)                                            # [128, ci, C]
    bqk = np.concatenate([bq, bk], axis=0).reshape(128, 1)
    bvr = bv.reshape(1, C)

    in_maps = []
    for b in range(NCORES):
        xb = np.ascontiguousarray(
            x[b].reshape(NCH, 128, NTB, TB).transpose(1, 2, 0, 3)
        ).astype(bf16)                                        # [128, tq, ci, TB]
        in_maps.append(
            {"x": xb, "wqkT": wqkT, "wvT": wvT, "bqk": bqk, "bv": bvr}
        )
    return in_maps


def kernel(**inputs: np.ndarray) -> np.ndarray:
    nc = _get_program()
    in_maps = prep_inputs(inputs)
    res = run_bass_kernel_spmd(nc, in_maps, list(range(NCORES)))
    out = np.stack([res.results[b]["out"] for b in range(NCORES)], axis=0)
    return out.astype(np.float32)


if __name__ == "__main__":
    import reference

    inputs = {k: np.asarray(v) for k, v in reference.setup_inputs().items()}
    expected = np.asarray(reference.reference(**inputs))
    actual = kernel(**inputs)
    rel = np.linalg.norm(actual - expected) / np.linalg.norm(expected)
    print("Relative error:", rel)


# revision 20
# speedup vs baseline: 1.1814x; 1.0101x over previous
"""Trainium2 Bass kernel for per-batch channel attention (CxAM-style).

Reference (per batch element b):
    q = (Wq @ x_b + bq)        # [64, T]
    k = (Wk @ x_b + bk)        # [64, T]
    v = (Wv @ x_b + bv)        # [512, T]
    R = q.T @ k                # [T, T]
    A = softmax(R, axis=-1)
    out_b = v @ A.T            # [512, T]

Sharding: pure data-parallel - batch B=8, one batch element per NeuronCore.

Key layout decisions:
  - All weight transposes happen on HOST (numpy) - the device receives
    wqkT [ch,ci,qk-out] and wvT [ch,ci,c] ready for matmul, plus x
    pre-cast to bf16 and pre-tiled as [128, tq, ci, 512] so every DMA is
    128 fat contiguous descriptors. x in bf16 halves HBM traffic.
  - x streams in 4 t-slabs; QK and V projections chase the slabs.
  - qk packs Q(rows 0:64)/K(64:128); kq is the swapped copy built by
    ACT Identity+bias reads of the projection PSUM at shifted base.
  - Scores S^T[s,t] per 128-s-chunk (contract 64); exp on ACT into bf16.
  - A.T@V via accumulating matmuls: stationary vT chunk, moving exp tile.
  - Softmax denominator: ones[128,128] bf16 stationary matmuls reduce the
    exp tiles AND land the result pre-broadcast across partitions, so the
    epilogue is just reciprocal + multiply (no [1,N] ops, no broadcast
    matmul chain). Keeping DVE/GPSIMD traffic minimal matters: heavy
    concurrent engine activity slows every PE matmul ~15-25%.
"""

import os

os.environ.setdefault("MYCRO_LOCAL_CACHE", "1")

import numpy as np

import concourse.bass as bass
import concourse.mybir as mybir
import concourse.tile as tile
from concourse import bacc
from concourse.bass_utils import run_bass_kernel_spmd

F32 = mybir.dt.float32
F32R = mybir.dt.float32r
BF16 = mybir.dt.bfloat16
AF = mybir.ActivationFunctionType

B = 8
C = 512
T = 2048
CQ = 64
NCORES = 8

TB = 512            # t-block / slab width
NTB = T // TB       # 4
NSC = T // 128      # 16 s-chunks
NPAIR = NSC // 2    # 8 score pairs per t-block
NCH = C // 128      # 4 contraction chunks
NCC = C // 128      # 4 output channel chunks


def _build_program() -> bass.Bass:
    nc = bacc.Bacc("TRN2", target_bir_lowering=False, debug=False, num_devices=NCORES)

    x_d = nc.declare_dram_parameter("x", [128, NTB, NCH, TB], BF16, isOutput=False)
    wqkT_d = nc.declare_dram_parameter("wqkT", [128, NCH, 128], BF16, isOutput=False)
    wvT_d = nc.declare_dram_parameter("wvT", [128, NCH, C], BF16, isOutput=False)
    bqk_d = nc.declare_dram_parameter("bqk", [128, 1], F32, isOutput=False)
    bv_d = nc.declare_dram_parameter("bv", [1, C], F32, isOutput=False)
    out_d = nc.declare_dram_parameter("out", [C, T], F32, isOutput=True)

    with tile.TileContext(nc) as tc:
        with (
            tc.tile_pool(name="const", bufs=1) as const,
            tc.tile_pool(name="weights", bufs=1) as wpool,
        ):
            # ---- constants
            onesr_f = const.tile([1, 128], F32)
            nc.gpsimd.memset(onesr_f[:], 1.0)
            ones_row = const.tile([1, 128], F32R)
            nc.vector.tensor_copy(ones_row[:], onesr_f[:])
            ones_bb = const.tile([128, 128], BF16)
            nc.gpsimd.memset(ones_bb[:], 1.0)

            # ---- inputs -> SBUF (x streamed per t-slab; slab 0 issued
            # first so its transfer starts before the small params queue)
            x_bf = wpool.tile([128, NTB, NCH, TB], BF16)
            wvT = wpool.tile([128, NCH, C], BF16)
            wqkT = wpool.tile([128, NCH, 128], BF16)
            bqk = wpool.tile([128, 1], F32)
            bv_row = wpool.tile([1, C], F32)
            nc.sync.dma_start(out=x_bf[:, 0], in_=x_d[:, 0])
            nc.sync.dma_start(out=wqkT[:], in_=wqkT_d[:])
            nc.sync.dma_start(out=bqk[:], in_=bqk_d[:])
            nc.sync.dma_start(out=bv_row[:], in_=bv_d[:])
            nc.sync.dma_start(out=wvT[:], in_=wvT_d[:])
            for tq in range(1, NTB):
                nc.sync.dma_start(out=x_bf[:, tq], in_=x_d[:, tq])

            qk = wpool.tile([128, T], BF16)   # rows 0:64 Q, 64:128 K
            kq = wpool.tile([128, T], BF16)   # rows 0:64 K, 64:128 Q
            vT = wpool.tile([128, NSC, C], BF16)
            bv_bcast = wpool.tile([128, C], F32)

            with (
                tc.tile_pool(name="ps_proj", bufs=1, space="PSUM") as ps_proj,
            ):
                bv_r = wpool.tile([1, C], F32R)
                nc.vector.tensor_copy(bv_r[:], bv_row[:])

                for tq in range(NTB):
                    tsl = slice(tq * TB, (tq + 1) * TB)
                    # packed Q/K projection for this slab
                    ps = ps_proj.tile([128, TB], F32, tag="qkp", bufs=2, name=f"qkp_{tq}")
                    for ci in range(NCH):
                        nc.tensor.matmul(
                            ps[:],
                            wqkT[:, ci, :],
                            x_bf[:, tq, ci, :],
                            start=(ci == 0),
                            stop=(ci == NCH - 1),
                        )
                    nc.scalar.activation(qk[:, tsl], ps[:], AF.Identity, bias=bqk[:, 0:1])
                    # swapped copy: kq rows 0:64 = K, 64:128 = Q
                    nc.scalar.activation(kq[0:CQ, tsl], ps[CQ:128, :], AF.Identity, bias=bqk[CQ:128, 0:1])
                    nc.scalar.activation(kq[CQ:128, tsl], ps[0:CQ, :], AF.Identity, bias=bqk[0:CQ, 0:1])

                    if tq == 0:
                        # bv broadcast [1,C] -> [128,C] via f32r ones matmul,
                        # tucked behind the slab-0 QK projection
                        bvb = ps_proj.tile([128, C], F32, tag="bvb", bufs=1)
                        nc.tensor.matmul(bvb[:], ones_row[:], bv_r[:], start=True, stop=True)
                        nc.vector.tensor_copy(bv_bcast[:], bvb[:])

                    # V^T projection for the 4 s-chunks of this slab
                    for jj in range(NCH):
                        j = NCH * tq + jj
                        psv = ps_proj.tile([128, C], F32, tag="vp", bufs=4, name=f"vp_{j}")
                        for ci in range(NCH):
                            nc.tensor.matmul(
                                psv[:],
                                x_bf[:, tq, ci, jj * 128:(jj + 1) * 128],
                                wvT[:, ci, :],
                                start=(ci == 0),
                                stop=(ci == NCH - 1),
                            )
                        nc.vector.tensor_add(vT[:, j, :], psv[:], bv_bcast[:])

            # ---- main attention loop, software-pipelined one pair deep
            with (
                tc.tile_pool(name="et", bufs=6) as et_pool,
                tc.tile_pool(name="ps_sc", bufs=1, space="PSUM") as ps_sc,
                tc.tile_pool(name="ps_av", bufs=1, space="PSUM") as ps_av,
                tc.tile_pool(name="ps_dn", bufs=1, space="PSUM") as ps_dn,
                tc.tile_pool(name="small", bufs=2) as small,
                tc.tile_pool(name="outp", bufs=2) as outp,
            ):
                avs = {}
                dnbs = {}

                def start_block(tb):
                    avs[tb] = [
                        ps_av.tile([128, TB], F32, tag=f"av{ck}", name=f"av{ck}_{tb}")
                        for ck in range(NCC)
                    ]
                    dnbs[tb] = ps_dn.tile([128, TB], F32, tag="dnb", name=f"dnb_{tb}")

                def emit_scores(tb, jp):
                    tsl = slice(tb * TB, (tb + 1) * TB)
                    j0, j1 = 2 * jp, 2 * jp + 1
                    etp0 = et_pool.tile([128, TB], BF16, tag="etp0", name=f"etp0_{tb}_{jp}")
                    etp1 = et_pool.tile([128, TB], BF16, tag="etp1", name=f"etp1_{tb}_{jp}")
                    sc0 = ps_sc.tile([128, TB], F32, tag="sc0", bufs=2, name=f"sc0_{tb}_{jp}")
                    nc.tensor.matmul(
                        sc0[:],
                        kq[0:CQ, j0 * 128:(j0 + 1) * 128],
                        qk[0:CQ, tsl],
                        start=True,
                        stop=True,
                    )
                    sc1 = ps_sc.tile([128, TB], F32, tag="sc1", name=f"sc1_{tb}_{jp}")
                    nc.tensor.matmul(
                        sc1[:],
                        qk[CQ:128, j1 * 128:(j1 + 1) * 128],
                        kq[CQ:128, tsl],
                        start=True,
                        stop=True,
                        tile_position=(64, 0),
                    )
                    nc.scalar.activation(etp0[:], sc0[:], AF.Exp)
                    nc.scalar.activation(etp1[:], sc1[:], AF.Exp)
                    return (etp0, etp1)

                def emit_consume(tb, jp, etp):
                    for idx in (0, 1):
                        j = 2 * jp + idx
                        # denominator: ones[128,128] stationary makes the
                        # reduction land pre-broadcast across partitions
                        nc.tensor.matmul(
                            dnbs[tb][:],
                            ones_bb[:],
                            etp[idx][:],
                            start=(j == 0),
                            stop=(j == NSC - 1),
                        )
                        for ck in range(NCC):
                            nc.tensor.matmul(
                                avs[tb][ck][:],
                                vT[:, j, ck * 128:(ck + 1) * 128],
                                etp[idx][:],
                                start=(j == 0),
                                stop=(j == NSC - 1),
                            )

                def finish_block(tb):
                    tsl = slice(tb * TB, (tb + 1) * TB)
                    rcp = small.tile([128, TB], F32, tag="rcp", name=f"rcp_{tb}")
                    nc.vector.reciprocal_approx_fast(rcp[:], dnbs[tb][:])

                    for ck in range(NCC):
                        ot = outp.tile(
                            [128, TB], F32, tag=f"ot{ck}", name=f"ot{ck}_{tb}"
                        )
                        nc.vector.tensor_mul(ot[:], avs[tb][ck][:], rcp[:])
                        nc.sync.dma_start(
                            out=out_d[ck * 128:(ck + 1) * 128, tsl], in_=ot[:]
                        )

                pending = None  # (tb, jp, etp)
                for tb in range(NTB):
                    for jp in range(NPAIR):
                        etp = emit_scores(tb, jp)
                        if jp == 0:
                            # after the first scores: sc banks get the
                            # earliest-freed projection banks
                            start_block(tb)
                        if pending is not None:
                            ptb, pjp, petp = pending
                            emit_consume(ptb, pjp, petp)
                            if pjp == NPAIR - 1:
                                finish_block(ptb)
                        pending = (tb, jp, etp)
                ptb, pjp, petp = pending
                emit_consume(ptb, pjp, petp)
                finish_block(ptb)

    nc.compile()
    return nc


_PROGRAM = None


def _get_program() -> bass.Bass:
    global _PROGRAM
    if _PROGRAM is None:
        _PROGRAM = _build_program()
    return _PROGRAM


def prep_inputs(inputs):
    """Host-side packing: transpose/tile weights, cast x to bf16 slabs."""
    import ml_dtypes

    bf16 = ml_dtypes.bfloat16
    x = np.asarray(inputs["x"], dtype=np.float32)
    wq = np.asarray(inputs["Wq"], dtype=np.float32)
    bq = np.asarray(inputs["bq"], dtype=np.float32)
    wk = np.asarray(inputs["Wk"], dtype=np.float32)
    bk = np.asarray(inputs["bk"], dtype=np.float32)
    wv = np.asarray(inputs["Wv"], dtype=np.float32)
    bv = np.asarray(inputs["bv"], dtype=np.float32)

    w8 = np.concatenate([wq, wk], axis=0)                     # [128, C]
    wqkT = np.ascontiguousarray(
        w8.T.reshape(NCH, 128, 128).transpose(1, 0, 2)
    ).astype(bf16)                                            # [128, ci, 128]
    wvT = np.ascontiguousarray(
        wv.T.reshape(NCH, 128, C).transpose(1, 0, 2)
    ).astype(bf16)                                            # [128, ci, C]
    bqk = np.concatenate([bq, bk], axis=0).reshape(128, 1)
    bvr = bv.reshape(1, C)

    in_maps = []
    for b in range(NCORES):
        xb = np.ascontiguousarray(
            x[b].reshape(NCH, 128, NTB, TB).transpose(1, 2, 0, 3)
        ).astype(bf16)                                        # [128, tq, ci, TB]
        in_maps.append(
            {"x": xb, "wqkT": wqkT, "wvT": wvT, "bqk": bqk, "bv": bvr}
        )
    return in_maps


def kernel(**inputs: np.ndarray) -> np.ndarray:
    nc = _get_program()
    in_maps = prep_inputs(inputs)
    res = run_bass_kernel_spmd(nc, in_maps, list(range(NCORES)))
    out = np.stack([res.results[b]["out"] for b in range(NCORES)], axis=0)
    return out.astype(np.float32)


if __name__ == "__main__":
    import reference

    inputs = {k: np.asarray(v) for k, v in reference.setup_inputs().items()}
    expected = np.asarray(reference.reference(**inputs))
    actual = kernel(**inputs)
    rel = np.linalg.norm(actual - expected) / np.linalg.norm(expected)
    print("Relative error:", rel)
